# revision 10
# baseline (speedup 1.0000x reference)
"""EnergyPool2d Trainium2 kernel.

For each 3x3 sliding window (stride 1, no padding) of each (n,c) image
plane, the reference scatter-adds +1 at the window's argmax position and
-1 at the argmin position (first-occurrence, row-major within window).

Separable reformulation (no scatter):
  S[i,v]   = max(x[i,v], x[i,v+1], x[i,v+2])          horizontal 3-max
  window (u,v) max = max(S[u,v], S[u+1,v], S[u+2,v])
  winner row  = first i in {u,u+1,u+2} with S[i,v] == window max
  winner col  = first j in {v,v+1,v+2} with x[i,j] == S[i,v]
  (this is exactly row-major first-occurrence argmax)

With C[u] = S[u]>=S[u+1], D[u] = S[u]>=S[u+2] (exactly one V_a is 1):
  V_0[u] = C[u]*D[u],  V_1[u] = (1-C[u])*C[u+1],  V_2[u] = (1-D[u])*(1-C[u+1])
  T[i]   = V_0[i] + V_1[i-1] + V_2[i-2]   # windows won by row i
  counts[i, v+b] += (x[i,v+b] == S[i,v]) * T[i,v]     for b = 0,1,2

Min path identical with min / <= and -1 accumulation.

Compute-engine access patterns must start at quadrant partitions (0/32/64/96),
so partition(row)-shifted operands are materialized with SBUF->SBUF DMA copies
(DMA is partition-unconstrained); edge partitions keep filler values (+-1e30 /
1.0) that are memset once into persistent tiles.

Data-parallel: 1024 (n,c) planes, 128 per core, 8 cores.
"""

import numpy as np

import concourse.bacc as bacc
import concourse.tile as tile
import concourse.mybir as mybir
from concourse import bass_utils

N_, C_, H, W = 16, 64, 128, 128
NCORES = 8
PLANES_PER_CORE = N_ * C_ // NCORES  # 128
TP = 16                              # planes per SBUF tile
NTILES = PLANES_PER_CORE // TP

F32 = mybir.dt.float32
BF16 = mybir.dt.bfloat16
Alu = mybir.AluOpType

Wm2 = W - 2  # 126


def _emit_path(nc, pools, shifts, xt, cnt, is_max: bool):
    """One (max or min) path for a plane tile.

    xt:  [128, TP, W] f32;  cnt: [128, TP, W] bf16 accumulator.
    shifts: persistent row-shift landing tiles (edge rows pre-filled).
    """
    v = nc.vector
    cmp3 = Alu.max if is_max else Alu.min
    cmpge = Alu.is_ge if is_max else Alu.is_le
    sfx = "m" if is_max else "n"
    bf = pools["bf"]
    S1, S2, C1, D2 = shifts[sfx]

    S = pools["f32"].tile([128, TP, Wm2], F32, tag="S" + sfx)
    # S = 3-max/min along W
    v.tensor_tensor(S[:], xt[:, :, 0:Wm2], xt[:, :, 1 : W - 1], cmp3)
    v.tensor_tensor(S[:], S[:], xt[:, :, 2:W], cmp3)

    # row-shifted copies: S1[i] = S[i+1], S2[i] = S[i+2]; persistent filler at
    # the tail rows makes the >=/<= compares there come out 0.
    nc.sync.dma_start(S1[0:127], S[1:128])
    nc.sync.dma_start(S2[0:126], S[2:128])

    Ct = bf.tile([128, TP, Wm2], BF16, tag="C" + sfx)
    Dt = bf.tile([128, TP, Wm2], BF16, tag="D" + sfx)
    v.tensor_tensor(Ct[:], S[:], S1[:], cmpge)   # C[127] = 0 via filler
    v.tensor_tensor(Dt[:], S[:], S2[:], cmpge)   # D[126:128] = 0 via filler

    # C1[i] = C[i-1] (row 0 filler 1.0), D2[i] = D[i-2] (rows 0,1 filler 1.0)
    nc.sync.dma_start(C1[1:128], Ct[0:127])
    nc.sync.dma_start(D2[2:128], Dt[0:126])

    nC1 = bf.tile([128, TP, Wm2], BF16, tag="nC1" + sfx)
    v.tensor_scalar_sub(nC1[:], C1[:], 1.0)      # nC1 = C[i-1]-1 (0 at row 0)

    # T = V_0 + V_1[i-1] + V_2[i-2]
    #   = C*D - nC1*C + (D2-1)*nC1
    T = bf.tile([128, TP, Wm2], BF16, tag="T" + sfx)
    t1 = bf.tile([128, TP, Wm2], BF16, tag="t1" + sfx)
    v.tensor_tensor(T[:], Ct[:], Dt[:], Alu.mult)
    v.tensor_tensor(t1[:], nC1[:], Ct[:], Alu.mult)
    v.tensor_tensor(T[:], T[:], t1[:], Alu.subtract)
    t2 = bf.tile([128, TP, Wm2], BF16, tag="t2" + sfx)
    nc.vector.scalar_tensor_tensor(t2[:], D2[:], 1.0, nC1[:], Alu.subtract, Alu.mult)
    v.tensor_tensor(T[:], T[:], t2[:], Alu.add)

    # e_b = (x[:, v+b] == S[:, v]);  counts[:, v+b] +-= e_b * T
    accop = Alu.add if is_max else Alu.subtract
    for b in range(3):
        e = bf.tile([128, TP, Wm2], BF16, tag="e" + sfx, bufs=2)
        v.tensor_tensor(e[:], xt[:, :, b : b + Wm2], S[:], Alu.is_equal)
        g = bf.tile([128, TP, Wm2], BF16, tag="g" + sfx, bufs=2)
        v.tensor_tensor(g[:], e[:], T[:], Alu.mult)
        v.tensor_tensor(cnt[:, :, b : b + Wm2], cnt[:, :, b : b + Wm2], g[:], accop)


def _emit_tiles(tc, nc, pools, shifts, x_ap, y_ap):
    io_pool = pools["io"]
    for t in range(NTILES):
        xt = io_pool.tile([128, TP, W], F32, tag="xt")
        # DRAM [TP, H, W] -> SBUF [H, TP, W]
        src = x_ap[t * TP : (t + 1) * TP].rearrange("p h w -> h p w")
        nc.sync.dma_start(xt[:], src)

        cnt = io_pool.tile([128, TP, W], BF16, tag="cnt")
        nc.gpsimd.memset(cnt[:], 0.0)

        _emit_path(nc, pools, shifts, xt, cnt, True)
        _emit_path(nc, pools, shifts, xt, cnt, False)

        out = io_pool.tile([128, TP, W], F32, tag="out")
        nc.scalar.copy(out[:], cnt[:])
        dst = y_ap[t * TP : (t + 1) * TP].rearrange("p h w -> h p w")
        nc.sync.dma_start(dst, out[:])


def _emit_kernel(tc, x_ap, y_ap, repeat=1):
    nc = tc.nc
    pools = {}
    with (
        tc.tile_pool(name="io", bufs=2) as io_pool,
        tc.tile_pool(name="f32", bufs=2) as f32_pool,
        tc.tile_pool(name="bf", bufs=1) as bf_pool,
        tc.tile_pool(name="sh", bufs=1) as sh_pool,
    ):
        pools["io"] = io_pool
        pools["f32"] = f32_pool
        pools["bf"] = bf_pool

        # persistent shift-landing tiles; fill edge rows once
        shifts = {}
        for sfx, fill in (("m", 1e30), ("n", -1e30)):
            S1 = sh_pool.tile([128, TP, Wm2], F32, tag="S1" + sfx)
            S2 = sh_pool.tile([128, TP, Wm2], F32, tag="S2" + sfx)
            C1 = sh_pool.tile([128, TP, Wm2], BF16, tag="C1" + sfx)
            D2 = sh_pool.tile([128, TP, Wm2], BF16, tag="D2" + sfx)
            nc.gpsimd.memset(S1[96:128], fill)
            nc.gpsimd.memset(S2[96:128], fill)
            nc.gpsimd.memset(C1[0:32], 1.0)
            nc.gpsimd.memset(D2[0:32], 1.0)
            shifts[sfx] = (S1, S2, C1, D2)

        if repeat == 1:
            _emit_tiles(tc, nc, pools, shifts, x_ap, y_ap)
        else:
            # timing amplification: HW loop re-running the identical
            # (idempotent) computation `repeat` times
            with tc.For_i(0, repeat, 1):
                _emit_tiles(tc, nc, pools, shifts, x_ap, y_ap)


_NC_CACHE = {}


def _build(repeat=1):
    if repeat in _NC_CACHE:
        return _NC_CACHE[repeat]
    nc = bacc.Bacc(
        "TRN2",
        target_bir_lowering=False,
        debug=False,
        enable_asserts=False,
        num_devices=NCORES,
    )
    x_d = nc.dram_tensor("x", [PLANES_PER_CORE, H, W], F32, kind="ExternalInput")
    y_d = nc.dram_tensor("y", [PLANES_PER_CORE, H, W], F32, kind="ExternalOutput")
    with tile.TileContext(nc) as tc:
        _emit_kernel(tc, x_d.ap(), y_d.ap(), repeat=repeat)
    nc.compile()
    _NC_CACHE[repeat] = nc
    return nc


def run(x, **spmd_kwargs):
    """Run on 8 cores; returns (full output [16,64,128,128], BassKernelResults)."""
    x = np.ascontiguousarray(np.asarray(x, dtype=np.float32))
    xf = x.reshape(N_ * C_, H, W)
    nc = _build()
    in_maps = [
        {"x": xf[k * PLANES_PER_CORE : (k + 1) * PLANES_PER_CORE]}
        for k in range(NCORES)
    ]
    res = bass_utils.run_bass_kernel_spmd(
        nc, in_maps, core_ids=list(range(NCORES)), **spmd_kwargs
    )
    out = np.concatenate([res.results[k]["y"] for k in range(NCORES)], axis=0)
    return out.reshape(N_, C_, H, W), res


def kernel(x):
    out, _ = run(x)
    return out


# revision 17
# speedup vs baseline: 1.6092x; 1.6092x over previous
"""EnergyPool2d Trainium2 kernel.

For each 3x3 sliding window (stride 1, no padding) of each (n,c) image
plane, the reference scatter-adds +1 at the window's argmax position and
-1 at the argmin position (first-occurrence, row-major within window).

Separable reformulation (no scatter):
  S[i,v]   = max(x[i,v], x[i,v+1], x[i,v+2])          horizontal 3-max
  window (u,v) max = max(S[u,v], S[u+1,v], S[u+2,v])
  winner row  = first i in {u,u+1,u+2} with S[i,v] == window max
  winner col  = first j in {v,v+1,v+2} with x[i,j] == S[i,v]
  (this is exactly row-major first-occurrence argmax)

Per-row window-win count T via adjacent-row compares of S:
  C[i] = S[i] >= S[i+1]        (wins forward pair)
  D[i] = S[i] >= S[i+2]
  P[i] = S[i] >  S[i-1]        (strictly beats backward)
  Q[i] = S[i] >  S[i-2]
  T[i] = C*D + P*(C + Q)       # = V0[i] + V1[i-1] + V2[i-2]
  counts[i, v+b] += (x[i,v+b] == S[i,v]) * T[i,v]   for b = 0,1,2

Min path identical with min / <= / < and -1 accumulation.

Compute engines cannot use partition(row)-shifted access patterns (quadrant
start constraint), so the row-shifted S fields are recomputed from row-shifted
copies of x that are DMA-loaded straight from DRAM (cheap).  Edge rows of the
shifted x copies hold NaN (set once; IEEE compares with NaN are false for
>=, <=, >, <, so the masks come out 0 exactly where no window exists).

Data-parallel: 1024 (n,c) planes, 128 per core, 8 cores.
"""

import numpy as np

import concourse.bacc as bacc
import concourse.tile as tile
import concourse.mybir as mybir
from concourse import bass_utils

N_, C_, H, W = 16, 64, 128, 128
NCORES = 8
PLANES_PER_CORE = N_ * C_ // NCORES  # 128
TP = 8                               # planes per SBUF tile
NTILES = PLANES_PER_CORE // TP

F32 = mybir.dt.float32
BF16 = mybir.dt.bfloat16
Alu = mybir.AluOpType

Wm2 = W - 2  # 126


def _max3(v, out, xt, op):
    v.tensor_tensor(out[:], xt[:, :, 0:Wm2], xt[:, :, 1 : W - 1], op)
    v.tensor_tensor(out[:], out[:], xt[:, :, 2:W], op)


def _emit_path(nc, pools, xts, cnt, is_max: bool):
    """One (max or min) path for a plane tile.

    xts: (xt, xd1, xd2, xu1, xu2) f32 [128, TP, W] tiles: x and its row-shifted
         copies (xd1[i] = x[i+1] ... xu2[i] = x[i-2], NaN at invalid rows).
    cnt: [128, TP, W] bf16 accumulator.
    """
    v = nc.vector
    cmp3 = Alu.max if is_max else Alu.min
    ge = Alu.is_ge if is_max else Alu.is_le
    gt = Alu.is_gt if is_max else Alu.is_lt
    sfx = "m" if is_max else "n"
    bf = pools["bf"]
    f32 = pools["f32"]
    xt, xd1, xd2, xu1, xu2 = xts

    S = f32.tile([128, TP, Wm2], F32, tag="S" + sfx)
    S1 = f32.tile([128, TP, Wm2], F32, tag="S1" + sfx)
    S2 = f32.tile([128, TP, Wm2], F32, tag="S2" + sfx)
    U1 = f32.tile([128, TP, Wm2], F32, tag="U1" + sfx)
    U2 = f32.tile([128, TP, Wm2], F32, tag="U2" + sfx)
    _max3(v, S, xt, cmp3)    # S[i]  valid all rows
    _max3(v, S1, xd1, cmp3)  # S[i+1], NaN at row 127
    _max3(v, S2, xd2, cmp3)  # S[i+2], NaN at rows 126,127
    _max3(v, U1, xu1, cmp3)  # S[i-1], NaN at row 0
    _max3(v, U2, xu2, cmp3)  # S[i-2], NaN at rows 0,1

    Ct = bf.tile([128, TP, Wm2], BF16, tag="C" + sfx)
    Dt = bf.tile([128, TP, Wm2], BF16, tag="D" + sfx)
    Pt = bf.tile([128, TP, Wm2], BF16, tag="P" + sfx)
    Qt = bf.tile([128, TP, Wm2], BF16, tag="Q" + sfx)
    v.tensor_tensor(Ct[:], S[:], S1[:], ge)
    v.tensor_tensor(Dt[:], S[:], S2[:], ge)
    v.tensor_tensor(Pt[:], S[:], U1[:], gt)
    v.tensor_tensor(Qt[:], S[:], U2[:], gt)

    # T = C*D + P*(C+Q)
    T = bf.tile([128, TP, Wm2], BF16, tag="T" + sfx)
    z = bf.tile([128, TP, Wm2], BF16, tag="z" + sfx)
    v.tensor_tensor(z[:], Ct[:], Qt[:], Alu.add)
    v.tensor_tensor(z[:], Pt[:], z[:], Alu.mult)
    v.tensor_tensor(T[:], Ct[:], Dt[:], Alu.mult)
    v.tensor_tensor(T[:], T[:], z[:], Alu.add)

    # e_b = (x[:, v+b] == S[:, v]);  counts[:, v+b] +-= e_b * T
    accop = Alu.add if is_max else Alu.subtract
    for b in range(3):
        e = bf.tile([128, TP, Wm2], BF16, tag="e" + sfx, bufs=2)
        v.tensor_tensor(e[:], xt[:, :, b : b + Wm2], S[:], Alu.is_equal)
        g = bf.tile([128, TP, Wm2], BF16, tag="g" + sfx, bufs=2)
        v.tensor_tensor(g[:], e[:], T[:], Alu.mult)
        v.tensor_tensor(cnt[:, :, b : b + Wm2], cnt[:, :, b : b + Wm2], g[:], accop)


def _emit_tiles(tc, nc, pools, x_ap, y_ap):
    io_pool = pools["io"]
    for t in range(NTILES):
        # x_ap is the NaN-padded input [TP planes, 132 rows, W]; real plane
        # row h lives at padded row h+2, NaN rows at 0,1,130,131.
        pl = x_ap[t * TP : (t + 1) * TP]
        xts = []
        for tag, r0 in (("xt", 2), ("xd1", 3), ("xd2", 4), ("xu1", 1), ("xu2", 0)):
            tl = io_pool.tile([128, TP, W], F32, tag=tag)
            # DRAM [TP, 128, W] slice -> SBUF [128, TP, W]
            nc.sync.dma_start(
                tl[:], pl[:, r0 : r0 + 128].rearrange("p h w -> h p w")
            )
            xts.append(tl)
        xt, xd1, xd2, xu1, xu2 = xts

        cnt = io_pool.tile([128, TP, W], BF16, tag="cnt")
        nc.gpsimd.memset(cnt[:], 0.0)

        _emit_path(nc, pools, xts, cnt, True)
        _emit_path(nc, pools, xts, cnt, False)

        out = io_pool.tile([128, TP, W], F32, tag="out")
        nc.scalar.copy(out[:], cnt[:])
        dst = y_ap[t * TP : (t + 1) * TP].rearrange("p h w -> h p w")
        nc.sync.dma_start(dst, out[:])


def _emit_kernel(tc, x_ap, y_ap, repeat=1):
    nc = tc.nc
    pools = {}
    with (
        tc.tile_pool(name="io", bufs=2) as io_pool,
        tc.tile_pool(name="f32", bufs=1) as f32_pool,
        tc.tile_pool(name="bf", bufs=1) as bf_pool,
    ):
        pools["io"] = io_pool
        pools["f32"] = f32_pool
        pools["bf"] = bf_pool

        if repeat == 1:
            _emit_tiles(tc, nc, pools, x_ap, y_ap)
        else:
            # timing amplification: HW loop re-running the identical
            # (idempotent) computation `repeat` times
            with tc.For_i(0, repeat, 1):
                _emit_tiles(tc, nc, pools, x_ap, y_ap)


_NC_CACHE = {}


def _build(repeat=1):
    if repeat in _NC_CACHE:
        return _NC_CACHE[repeat]
    nc = bacc.Bacc(
        "TRN2",
        target_bir_lowering=False,
        debug=False,
        enable_asserts=False,
        num_devices=NCORES,
    )
    x_d = nc.dram_tensor("x", [PLANES_PER_CORE, H + 4, W], F32, kind="ExternalInput")
    y_d = nc.dram_tensor("y", [PLANES_PER_CORE, H, W], F32, kind="ExternalOutput")
    with tile.TileContext(nc) as tc:
        _emit_kernel(tc, x_d.ap(), y_d.ap(), repeat=repeat)
    nc.compile()
    _NC_CACHE[repeat] = nc
    return nc


def make_in_maps(x):
    x = np.asarray(x, dtype=np.float32)
    xf = x.reshape(N_ * C_, H, W)
    # pad 2 NaN rows above and below each plane so the kernel's row-shifted
    # loads read NaN at out-of-plane rows (NaN compares false for >=/<=/>/<)
    xpad = np.full((N_ * C_, H + 4, W), np.nan, dtype=np.float32)
    xpad[:, 2 : H + 2] = xf
    return [
        {"x": xpad[k * PLANES_PER_CORE : (k + 1) * PLANES_PER_CORE]}
        for k in range(NCORES)
    ]


def run(x, **spmd_kwargs):
    """Run on 8 cores; returns (full output [16,64,128,128], BassKernelResults)."""
    nc = _build()
    in_maps = make_in_maps(x)
    res = bass_utils.run_bass_kernel_spmd(
        nc, in_maps, core_ids=list(range(NCORES)), **spmd_kwargs
    )
    out = np.concatenate([res.results[k]["y"] for k in range(NCORES)], axis=0)
    return out.reshape(N_, C_, H, W), res


def kernel(x):
    out, _ = run(x)
    return out


# revision 20
# speedup vs baseline: 1.6596x; 1.0313x over previous
"""EnergyPool2d Trainium2 kernel.

For each 3x3 sliding window (stride 1, no padding) of each (n,c) image
plane, the reference scatter-adds +1 at the window's argmax position and
-1 at the argmin position (first-occurrence, row-major within window).

Separable reformulation (no scatter):
  S[i,v]   = max(x[i,v], x[i,v+1], x[i,v+2])          horizontal 3-max
  window (u,v) max = max(S[u,v], S[u+1,v], S[u+2,v])
  winner row  = first i in {u,u+1,u+2} with S[i,v] == window max
  winner col  = first j in {v,v+1,v+2} with x[i,j] == S[i,v]
  (this is exactly row-major first-occurrence argmax)

Per-row window-win count T via adjacent-row compares of S:
  C[i] = S[i] >= S[i+1]        (wins forward pair)
  D[i] = S[i] >= S[i+2]
  P[i] = S[i] >  S[i-1]        (strictly beats backward)
  Q[i] = S[i] >  S[i-2]
  T[i] = C*D + P*(C + Q)       # = V0[i] + V1[i-1] + V2[i-2]
  counts[i, v+b] += (x[i,v+b] == S[i,v]) * T[i,v]   for b = 0,1,2

Min path identical with min / <= / < and -1 accumulation.

Compute engines cannot use partition(row)-shifted access patterns (quadrant
start constraint), so the row-shifted S fields are recomputed from row-shifted
copies of x that are DMA-loaded straight from DRAM (cheap).  Edge rows of the
shifted x copies hold NaN (set once; IEEE compares with NaN are false for
>=, <=, >, <, so the masks come out 0 exactly where no window exists).

Data-parallel: 1024 (n,c) planes, 128 per core, 8 cores.
"""

import numpy as np

import concourse.bacc as bacc
import concourse.tile as tile
import concourse.mybir as mybir
from concourse import bass_utils

N_, C_, H, W = 16, 64, 128, 128
NCORES = 8
PLANES_PER_CORE = N_ * C_ // NCORES  # 128
TP = 16                              # planes per SBUF tile
NTILES = PLANES_PER_CORE // TP

F32 = mybir.dt.float32
BF16 = mybir.dt.bfloat16
Alu = mybir.AluOpType

Wm2 = W - 2  # 126


def _max3(v, out, xt, op):
    v.tensor_tensor(out[:], xt[:, :, 0:Wm2], xt[:, :, 1 : W - 1], op)
    v.tensor_tensor(out[:], out[:], xt[:, :, 2:W], op)


def _emit_path(nc, pools, xts, cnt, is_max: bool):
    """One (max or min) path for a plane tile.

    xts: (xt, xd1, xd2, xu1, xu2) f32 [128, TP, W] tiles: x and its row-shifted
         copies (xd1[i] = x[i+1] ... xu2[i] = x[i-2], NaN at invalid rows).
    cnt: [128, TP, W] bf16 accumulator.
    """
    v = nc.vector
    vs = nc.vector
    cmp3 = Alu.max if is_max else Alu.min
    ge = Alu.is_ge if is_max else Alu.is_le
    gt = Alu.is_gt if is_max else Alu.is_lt
    sfx = "m" if is_max else "n"
    bf = pools["bf"]
    f32 = pools["f32"]
    xt, xd1, xd2, xu1, xu2 = xts

    S = f32.tile([128, TP, Wm2], F32, tag="S")
    S1 = f32.tile([128, TP, Wm2], F32, tag="S1")
    S2 = f32.tile([128, TP, Wm2], F32, tag="S2")
    U1 = f32.tile([128, TP, Wm2], F32, tag="U1")
    U2 = f32.tile([128, TP, Wm2], F32, tag="U2")
    _max3(v, S, xt, cmp3)     # S[i]  valid all rows
    _max3(vs, S1, xd1, cmp3)  # S[i+1], NaN at row 127
    _max3(vs, S2, xd2, cmp3)  # S[i+2], NaN at rows 126,127
    _max3(vs, U1, xu1, cmp3)  # S[i-1], NaN at row 0
    _max3(vs, U2, xu2, cmp3)  # S[i-2], NaN at rows 0,1

    Ct = bf.tile([128, TP, Wm2], BF16, tag="C")
    Dt = bf.tile([128, TP, Wm2], BF16, tag="D")
    Pt = bf.tile([128, TP, Wm2], BF16, tag="P")
    Qt = bf.tile([128, TP, Wm2], BF16, tag="Q")
    vs.tensor_tensor(Ct[:], S[:], S1[:], ge)
    vs.tensor_tensor(Dt[:], S[:], S2[:], ge)
    vs.tensor_tensor(Pt[:], S[:], U1[:], gt)
    vs.tensor_tensor(Qt[:], S[:], U2[:], gt)

    # T = C*D + P*(C+Q)
    T = bf.tile([128, TP, Wm2], BF16, tag="T")
    z = bf.tile([128, TP, Wm2], BF16, tag="z")
    v.tensor_tensor(z[:], Ct[:], Qt[:], Alu.add)
    v.tensor_tensor(z[:], Pt[:], z[:], Alu.mult)
    v.tensor_tensor(T[:], Ct[:], Dt[:], Alu.mult)
    v.tensor_tensor(T[:], T[:], z[:], Alu.add)

    # e_b = (x[:, v+b] == S[:, v]);  counts[:, v+b] +-= e_b * T
    accop = Alu.add if is_max else Alu.subtract
    for b in range(3):
        e = bf.tile([128, TP, Wm2], BF16, tag="C")
        v.tensor_tensor(e[:], xt[:, :, b : b + Wm2], S[:], Alu.is_equal)
        g = bf.tile([128, TP, Wm2], BF16, tag="P")
        v.tensor_tensor(g[:], e[:], T[:], Alu.mult)
        v.tensor_tensor(cnt[:, :, b : b + Wm2], cnt[:, :, b : b + Wm2], g[:], accop)


def _emit_tiles(tc, nc, pools, x_ap, y_ap):
    io_pool = pools["io"]
    for t in range(NTILES):
        # x_ap is the NaN-padded input [TP planes, 132 rows, W]; real plane
        # row h lives at padded row h+2, NaN rows at 0,1,130,131.
        pl = x_ap[t * TP : (t + 1) * TP]
        xts = []
        for tag, r0 in (("xt", 2), ("xd1", 3), ("xd2", 4), ("xu1", 1), ("xu2", 0)):
            tl = io_pool.tile([128, TP, W], F32, tag=tag)
            # DRAM [TP, 128, W] slice -> SBUF [128, TP, W]
            nc.sync.dma_start(
                tl[:], pl[:, r0 : r0 + 128].rearrange("p h w -> h p w")
            )
            xts.append(tl)
        xt, xd1, xd2, xu1, xu2 = xts

        cnt = io_pool.tile([128, TP, W], BF16, tag="cnt")
        nc.gpsimd.memset(cnt[:], 0.0)

        _emit_path(nc, pools, xts, cnt, True)
        _emit_path(nc, pools, xts, cnt, False)

        out = io_pool.tile([128, TP, W], F32, tag="out")
        nc.scalar.copy(out[:], cnt[:])
        dst = y_ap[t * TP : (t + 1) * TP].rearrange("p h w -> h p w")
        nc.sync.dma_start(dst, out[:])


def _emit_kernel(tc, x_ap, y_ap, repeat=1):
    nc = tc.nc
    pools = {}
    with (
        tc.tile_pool(name="io", bufs=2) as io_pool,
        tc.tile_pool(name="f32", bufs=1) as f32_pool,
        tc.tile_pool(name="bf", bufs=1) as bf_pool,
    ):
        pools["io"] = io_pool
        pools["f32"] = f32_pool
        pools["bf"] = bf_pool

        if repeat == 1:
            _emit_tiles(tc, nc, pools, x_ap, y_ap)
        else:
            # timing amplification: HW loop re-running the identical
            # (idempotent) computation `repeat` times
            with tc.For_i(0, repeat, 1):
                _emit_tiles(tc, nc, pools, x_ap, y_ap)


_NC_CACHE = {}


def _build(repeat=1):
    if repeat in _NC_CACHE:
        return _NC_CACHE[repeat]
    nc = bacc.Bacc(
        "TRN2",
        target_bir_lowering=False,
        debug=False,
        enable_asserts=False,
        num_devices=NCORES,
    )
    x_d = nc.dram_tensor("x", [PLANES_PER_CORE, H + 4, W], F32, kind="ExternalInput")
    y_d = nc.dram_tensor("y", [PLANES_PER_CORE, H, W], F32, kind="ExternalOutput")
    with tile.TileContext(nc) as tc:
        _emit_kernel(tc, x_d.ap(), y_d.ap(), repeat=repeat)
    nc.compile()
    _NC_CACHE[repeat] = nc
    return nc


def make_in_maps(x):
    x = np.asarray(x, dtype=np.float32)
    xf = x.reshape(N_ * C_, H, W)
    # pad 2 NaN rows above and below each plane so the kernel's row-shifted
    # loads read NaN at out-of-plane rows (NaN compares false for >=/<=/>/<)
    xpad = np.full((N_ * C_, H + 4, W), np.nan, dtype=np.float32)
    xpad[:, 2 : H + 2] = xf
    return [
        {"x": xpad[k * PLANES_PER_CORE : (k + 1) * PLANES_PER_CORE]}
        for k in range(NCORES)
    ]


def run(x, **spmd_kwargs):
    """Run on 8 cores; returns (full output [16,64,128,128], BassKernelResults)."""
    nc = _build()
    in_maps = make_in_maps(x)
    res = bass_utils.run_bass_kernel_spmd(
        nc, in_maps, core_ids=list(range(NCORES)), **spmd_kwargs
    )
    out = np.concatenate([res.results[k]["y"] for k in range(NCORES)], axis=0)
    return out.reshape(N_, C_, H, W), res


def kernel(x):
    out, _ = run(x)
    return out
